# revision 2
# baseline (speedup 1.0000x reference)
"""AttentionAggregator2d Trainium2 kernel v3 (8 NeuronCores, data-parallel).

v2 -> v3:
  - zm arrives as 4 big DMAs into one [128, 8192] buffer (top/bottom channel
    halves), issued BEFORE the zc load on the SWDGE queue, so the first k
    projection starts ~3us in (v2 queued 4MB of zc ahead of zm).
  - Stage-1 k/q are quad-packed: 8 matmuls -> one [128,1024] PSUM tile ->
    one contiguous FD=1024 evacuation (ACT for prefix k quads, DVE inside
    the exp stream).
  - Units run h-major (all half-0 t-tiles, then half-1), so the first score
    only needs k chunks 0..7 (quads 0-1); quads 2-3 drain as leftovers.
  - Leftover stage-1 work is placed on an explicit (run, unit) schedule that
    fits inside PE slack under the ACT-bound steady state.
"""

import numpy as np

N = 4096
C = 256
P = 32
TT = 128
RUN = 4
NTT = N // TT     # 32
NRUN = NTT // RUN # 8
IC = 512
PBLK = 12
B = 8

_cache = {}


def _build_module(repeat=1, lesion=None):
    import concourse.bacc as bacc
    import concourse.tile as tile
    from concourse import mybir
    from contextlib import ExitStack

    f32 = mybir.dt.float32
    bf16 = mybir.dt.bfloat16
    f16 = mybir.dt.float16
    AF = mybir.ActivationFunctionType
    OP = mybir.AluOpType

    nc = bacc.Bacc(trn_type="TRN2", debug=False)

    zm_d = nc.dram_tensor("zm", [C, N], f16, kind="ExternalInput").ap()
    zc_d = nc.dram_tensor("zc", [C, N], f32, kind="ExternalInput").ap()
    wq_d = nc.dram_tensor("wq4", [C, 128], f16, kind="ExternalInput").ap()
    wk_d = nc.dram_tensor("wk4", [C, 128], f16, kind="ExternalInput").ap()
    wv_d = nc.dram_tensor("wvt", [C, C], f16, kind="ExternalInput").ap()
    bq_d = nc.dram_tensor("bq4", [128, 1], f32, kind="ExternalInput").ap()
    bk_d = nc.dram_tensor("bk4", [128, 1], f32, kind="ExternalInput").ap()
    bv_d = nc.dram_tensor("bvr", [1, C], f16, kind="ExternalInput").ap()
    gam_d = nc.dram_tensor("gam", [128, 1], f32, kind="ExternalInput").ap()
    one_d = nc.dram_tensor("ones", [1, 128], f16, kind="ExternalInput").ap()
    out_d = nc.dram_tensor("out", [C, N], f32, kind="ExternalOutput").ap()

    with tile.TileContext(nc) as tc, ExitStack() as ctx:
        consts = ctx.enter_context(tc.tile_pool(name="consts", bufs=1))
        big = ctx.enter_context(tc.tile_pool(name="big", bufs=1))
        p_pool = ctx.enter_context(tc.tile_pool(name="pblk", bufs=PBLK))
        ut_pool = ctx.enter_context(tc.tile_pool(name="ut", bufs=PBLK))
        d_pool = ctx.enter_context(tc.tile_pool(name="dp", bufs=6))
        ps_s = ctx.enter_context(tc.tile_pool(name="ps_s", bufs=2, space="PSUM"))
        ps_c = ctx.enter_context(tc.tile_pool(name="ps_c", bufs=2, space="PSUM"))

        # ---- constants ----
        wq_sb = consts.tile([128, 256], f16, name="wq_sb")
        wk_sb = consts.tile([128, 256], f16, name="wk_sb")
        wv_sb = consts.tile([128, 512], f16, name="wv_sb")
        bq_sb = consts.tile([128, 1], f32, name="bq_sb")
        bk_sb = consts.tile([128, 1], f32, name="bk_sb")
        bv_sb = consts.tile([1, C], f16, name="bv_sb")
        one_sb = consts.tile([1, 128], f16, name="one_sb")
        gam_sb = consts.tile([128, 1], f32, name="gam_sb")
        exp_warm = consts.tile([1, 128], f32, name="exp_warm")
        for h in range(2):
            nc.sync.dma_start(out=wq_sb[:, h * 128:(h + 1) * 128],
                              in_=wq_d[h * 128:(h + 1) * 128, :])
            nc.sync.dma_start(out=wk_sb[:, h * 128:(h + 1) * 128],
                              in_=wk_d[h * 128:(h + 1) * 128, :])
            nc.sync.dma_start(out=wv_sb[:, h * 256:(h + 1) * 256],
                              in_=wv_d[h * 128:(h + 1) * 128, :])
        nc.sync.dma_start(out=bq_sb, in_=bq_d)
        nc.sync.dma_start(out=bk_sb, in_=bk_d)
        nc.sync.dma_start(out=bv_sb, in_=bv_d)
        nc.sync.dma_start(out=gam_sb, in_=gam_d)
        nc.sync.dma_start(out=one_sb, in_=one_d)

        # ---- persistent tiles ----
        q_rep = big.tile([128, N], f16, name="q_rep")
        k_rep = big.tile([128, N], f16, name="k_rep")
        vt = big.tile([128, NTT * C], bf16, name="vt")
        acc_sb = big.tile([128, 2 * N], f32, name="acc_sb")
        zmbig = big.tile([128, 2 * N], f16, name="zmbig")

        def zm_top(g):
            return zmbig[:, g * 256:(g + 1) * 256]

        def zm_bot(g):
            return zmbig[:, N + g * 256:N + (g + 1) * 256]

        for _rep in range(repeat):
          # ACT exp table preload, overlapping the DMAs
          nc.scalar.activation(exp_warm, one_sb, AF.Exp)
          # zm first (stage-1 needs it immediately), zc afterwards on SWDGE
          for hf in range(2):
            nc.sync.dma_start(out=zmbig[:, hf * 2048:(hf + 1) * 2048],
                              in_=zm_d[0:128, hf * 2048:(hf + 1) * 2048])
            nc.gpsimd.dma_start(out=zmbig[:, N + hf * 2048:N + (hf + 1) * 2048],
                                in_=zm_d[128:256, hf * 2048:(hf + 1) * 2048])
          for h in range(2):
            for qtr in range(2):
                nc.gpsimd.dma_start(
                    out=acc_sb[:, h * N + qtr * 2048:h * N + (qtr + 1) * 2048],
                    in_=zc_d[h * 128:(h + 1) * 128, qtr * 2048:(qtr + 1) * 2048])

          def emit_k_quad(qd, on_act):
            psk = ps_c.tile([128, 1024], f32, name="psk", tag="c")
            for j in range(4):
                g = 4 * qd + j
                nc.tensor.matmul(psk[:, j * 256:(j + 1) * 256], wk_sb[:, 0:128],
                                 zm_top(g), start=True, stop=False)
                nc.tensor.matmul(psk[:, j * 256:(j + 1) * 256], wk_sb[:, 128:256],
                                 zm_bot(g), start=False, stop=True)
            dst = k_rep[:, qd * 1024:(qd + 1) * 1024]
            if on_act:
                nc.scalar.activation(dst, psk, AF.Identity, bias=bk_sb)
            else:
                nc.vector.tensor_scalar_add(dst, psk, bk_sb)

          def emit_q_quad(qd):
            psq = ps_c.tile([128, 1024], f32, name="psq", tag="c")
            for j in range(4):
                g = 4 * qd + j
                nc.tensor.matmul(psq[:, j * 256:(j + 1) * 256], wq_sb[:, 0:128],
                                 zm_top(g), start=True, stop=False)
                nc.tensor.matmul(psq[:, j * 256:(j + 1) * 256], wq_sb[:, 128:256],
                                 zm_bot(g), start=False, stop=True)
            nc.vector.tensor_scalar_add(q_rep[:, qd * 1024:(qd + 1) * 1024],
                                        psq, bq_sb)

          def emit_v_run(r):
            psv = ps_c.tile([128, 1024], f32, name="psv", tag="c")
            for j in range(2):
                g = 2 * r + j
                for s in range(2):
                    o = j * 512 + s * 256
                    nc.tensor.matmul(psv[:, o:o + 256], one_sb,
                                     bv_sb, start=True, stop=False)
                    nc.tensor.matmul(psv[:, o:o + 256],
                                     zm_top(g)[:, s * 128:(s + 1) * 128],
                                     wv_sb[:, 0:256], start=False, stop=False)
                    nc.tensor.matmul(psv[:, o:o + 256],
                                     zm_bot(g)[:, s * 128:(s + 1) * 128],
                                     wv_sb[:, 256:512], start=False, stop=True)
            nc.vector.tensor_copy(vt[:, r * 4 * C:(r + 1) * 4 * C], psv)

          # prefix: k quads 0-1 (i 0..2047, enough for the h0 units), q quad 0
          emit_k_quad(0, True)
          emit_q_quad(0)
          emit_k_quad(1, True)

          # (run, unit) -> stage-1 leftovers, placed inside PE slack
          lsched = {
              (0, 0): [("k", 2)], (0, 1): [("k", 3)], (0, 2): [("v", 0)],
              (0, 3): [("v", 1)], (0, 4): [("q", 1)], (0, 5): [("v", 2)],
              (0, 6): [("v", 3)], (0, 7): [("q", 2)],
              (2, 2): [("v", 4)], (3, 2): [("v", 5)], (3, 6): [("q", 3)],
              (4, 2): [("v", 6)], (5, 2): [("v", 7)],
          }
          emitters = {"k": lambda g: emit_k_quad(g, False),
                      "q": emit_q_quad, "v": emit_v_run}

          pairs = [(c, kk) for kk in range(4) for c in range(2)]

          def emit_chain_pair(run, pts, uts, c, kk):
            a = ps_c.tile([128, 1024], f32, name="a_out", tag="c")
            for tl in range(RUN):
                for j in range(2):
                    nc.tensor.matmul(a[:, j * 512:(j + 1) * 512],
                                     uts[tl][:, c * 128:(c + 1) * 128],
                                     pts[tl][:, kk * 1024 + j * 512:
                                             kk * 1024 + (j + 1) * 512],
                                     start=(tl == 0), stop=(tl == RUN - 1))
            dst = acc_sb[:, c * N + kk * 1024:c * N + (kk + 1) * 1024]
            nc.vector.tensor_tensor(dst, a, dst, op=OP.add)
            if run == NRUN - 1:
                nc.gpsimd.dma_start(
                    out=out_d[c * 128:(c + 1) * 128, kk * 1024:(kk + 1) * 1024],
                    in_=dst)

          UNITS = [(0, 0), (1, 0), (2, 0), (3, 0), (0, 1), (1, 1), (2, 1), (3, 1)]

          prev = None
          for run in range(NRUN):
            pts = [None] * RUN
            uts = [None] * RUN
            dcol = d_pool.tile([128, 4 * RUN], f32, name="dcol", tag="dcol")
            for u, (tl, half) in enumerate(UNITS):
                tt = run * RUN + tl
                if half == 0:
                    pts[tl] = p_pool.tile([128, N], bf16, name="pt", tag="pt")
                for p in range(2):
                    sp = ps_s.tile([128, 1024], f32, name="s_sc", tag="s")
                    for r in range(2 * p, 2 * p + 2):
                        ic = half * 4 + r
                        nc.tensor.matmul(
                            sp[:, (r - 2 * p) * 512:(r - 2 * p + 1) * 512],
                            q_rep[32 * r:32 * (r + 1), tt * TT:(tt + 1) * TT],
                            k_rep[32 * r:32 * (r + 1), ic * IC:(ic + 1) * IC],
                            start=True, stop=True, tile_position=(32 * r, 0))
                    nc.scalar.activation(
                        pts[tl][:, half * 2048 + p * 1024:
                                half * 2048 + (p + 1) * 1024],
                        sp, AF.Exp,
                        accum_out=dcol[:, (tl * 2 + half) * 2 + p:
                                       (tl * 2 + half) * 2 + p + 1])
                if u in (5, 7):
                    hb = (u - 5) // 2
                    dview = dcol.rearrange("p (t h) -> p t h", h=4)
                    da = d_pool.tile([128, 2], f32, name="da", tag="da")
                    nc.vector.tensor_tensor(da, dview[:, 2 * hb:2 * hb + 2, 0],
                                            dview[:, 2 * hb:2 * hb + 2, 1],
                                            op=OP.add)
                    db = d_pool.tile([128, 2], f32, name="db", tag="db")
                    nc.vector.tensor_tensor(db, dview[:, 2 * hb:2 * hb + 2, 2],
                                            dview[:, 2 * hb:2 * hb + 2, 3],
                                            op=OP.add)
                    dsum = d_pool.tile([128, 2], f32, name="dsum", tag="dsum")
                    nc.vector.tensor_tensor(dsum, da, db, op=OP.add)
                    drec = d_pool.tile([128, 2], f32, name="drec", tag="drec")
                    nc.vector.reciprocal(drec, dsum)
                    for tl2 in (2 * hb, 2 * hb + 1):
                        tt2 = run * RUN + tl2
                        ut = ut_pool.tile([128, C], bf16, name="ut", tag="ut")
                        uts[tl2] = ut
                        nc.vector.tensor_scalar(ut, vt[:, tt2 * C:(tt2 + 1) * C],
                                                drec[:, tl2 - 2 * hb:tl2 - 2 * hb + 1],
                                                gam_sb, op0=OP.mult, op1=OP.mult)
                if prev is not None:
                    emit_chain_pair(run - 1, prev[0], prev[1], *pairs[u])
                for kind, g in lsched.get((run, u), []):
                    emitters[kind](g)
            prev = (pts, uts)
          for u in range(2 * RUN):
            emit_chain_pair(NRUN - 1, prev[0], prev[1], *pairs[u])

    nc.compile()
    return nc


def _get_module(repeat=1, lesion=None):
    key = f"nc{repeat}_{lesion}"
    if key not in _cache:
        _cache[key] = _build_module(repeat, lesion)
    return _cache[key]


def _host_prep(Wq, bq, Wk, bk, Wv, bv, gamma):
    g = np.float32(np.asarray(gamma).reshape(-1)[0])
    wq4 = np.ascontiguousarray(np.tile(np.asarray(Wq).T.astype(np.float16), (1, 4)))
    wk4 = np.ascontiguousarray(np.tile(np.asarray(Wk).T.astype(np.float16), (1, 4)))
    wvt = np.ascontiguousarray(np.asarray(Wv).T.astype(np.float16))
    bq4 = np.ascontiguousarray(np.tile(np.asarray(bq).astype(np.float32), 4).reshape(128, 1))
    bk4 = np.ascontiguousarray(np.tile(np.asarray(bk).astype(np.float32), 4).reshape(128, 1))
    bvr = np.ascontiguousarray(np.asarray(bv).astype(np.float16).reshape(1, C))
    gam = np.full((128, 1), g, np.float32)
    ones = np.ones((1, 128), np.float16)
    return dict(wq4=wq4, wk4=wk4, wvt=wvt, bq4=bq4, bk4=bk4, bvr=bvr, gam=gam, ones=ones)


def kernel(zc, zm, Wq, bq, Wk, bk, Wv, bv, gamma):
    from concourse.bass_utils import run_bass_kernel_spmd

    zc = np.asarray(zc)
    zm = np.asarray(zm)
    b, c, w, h = zm.shape
    assert (b, c, w * h) == (B, C, N), (zm.shape,)
    nc = _get_module()
    shared = _host_prep(Wq, bq, Wk, bk, Wv, bv, gamma)
    zmf = np.ascontiguousarray(zm.reshape(B, C, N).astype(np.float16))
    zcf = np.ascontiguousarray(zc.reshape(B, C, N).astype(np.float32))
    in_maps = [dict(zm=zmf[i], zc=zcf[i], **shared) for i in range(B)]
    res = run_bass_kernel_spmd(nc, in_maps, core_ids=list(range(B)))
    out = np.stack([r["out"] for r in res.results], axis=0)
    return out.reshape(b, c, w, h).astype(np.asarray(zc).dtype)


# revision 3
# speedup vs baseline: 1.2596x; 1.2596x over previous
"""AttentionAggregator2d Trainium2 kernel v3 (8 NeuronCores, data-parallel).

v2 -> v3:
  - zm arrives as 4 big DMAs into one [128, 8192] buffer (top/bottom channel
    halves), issued BEFORE the zc load on the SWDGE queue, so the first k
    projection starts ~3us in (v2 queued 4MB of zc ahead of zm).
  - Stage-1 k/q are quad-packed: 8 matmuls -> one [128,1024] PSUM tile ->
    one contiguous FD=1024 evacuation (ACT for prefix k quads, DVE inside
    the exp stream).
  - Units run h-major (all half-0 t-tiles, then half-1), so the first score
    only needs k chunks 0..7 (quads 0-1); quads 2-3 drain as leftovers.
  - Leftover stage-1 work is placed on an explicit (run, unit) schedule that
    fits inside PE slack under the ACT-bound steady state.
"""

import numpy as np

N = 4096
C = 256
P = 32
TT = 128
RUN = 4
NTT = N // TT     # 32
NRUN = NTT // RUN # 8
IC = 512
PBLK = 12
B = 8

_cache = {}


def _build_module(repeat=1, lesion=None):
    import concourse.bacc as bacc
    import concourse.tile as tile
    from concourse import mybir
    from contextlib import ExitStack

    f32 = mybir.dt.float32
    bf16 = mybir.dt.bfloat16
    f16 = mybir.dt.float16
    AF = mybir.ActivationFunctionType
    OP = mybir.AluOpType

    nc = bacc.Bacc(trn_type="TRN2", debug=False)

    zm_d = nc.dram_tensor("zm", [C, N], f16, kind="ExternalInput").ap()
    zc_d = nc.dram_tensor("zc", [C, N], f32, kind="ExternalInput").ap()
    wq_d = nc.dram_tensor("wq4", [C, 128], f16, kind="ExternalInput").ap()
    wk_d = nc.dram_tensor("wk4", [C, 128], f16, kind="ExternalInput").ap()
    wv_d = nc.dram_tensor("wvt", [C, C], f16, kind="ExternalInput").ap()
    bq_d = nc.dram_tensor("bq4", [128, 1], f32, kind="ExternalInput").ap()
    bk_d = nc.dram_tensor("bk4", [128, 1], f32, kind="ExternalInput").ap()
    bv_d = nc.dram_tensor("bvr", [1, C], f16, kind="ExternalInput").ap()
    gam_d = nc.dram_tensor("gam", [128, 1], f32, kind="ExternalInput").ap()
    one_d = nc.dram_tensor("ones", [1, 128], f16, kind="ExternalInput").ap()
    out_d = nc.dram_tensor("out", [C, N], f32, kind="ExternalOutput").ap()

    with tile.TileContext(nc) as tc, ExitStack() as ctx:
        consts = ctx.enter_context(tc.tile_pool(name="consts", bufs=1))
        big = ctx.enter_context(tc.tile_pool(name="big", bufs=1))
        p_pool = ctx.enter_context(tc.tile_pool(name="pblk", bufs=PBLK))
        ut_pool = ctx.enter_context(tc.tile_pool(name="ut", bufs=PBLK))
        d_pool = ctx.enter_context(tc.tile_pool(name="dp", bufs=6))
        ps_s = ctx.enter_context(tc.tile_pool(name="ps_s", bufs=2, space="PSUM"))
        ps_c = ctx.enter_context(tc.tile_pool(name="ps_c", bufs=2, space="PSUM"))

        # ---- constants ----
        wq_sb = consts.tile([128, 256], f16, name="wq_sb")
        wk_sb = consts.tile([128, 256], f16, name="wk_sb")
        wv_sb = consts.tile([128, 512], f16, name="wv_sb")
        bq_sb = consts.tile([128, 1], f32, name="bq_sb")
        bk_sb = consts.tile([128, 1], f32, name="bk_sb")
        bv_sb = consts.tile([1, C], f16, name="bv_sb")
        one_sb = consts.tile([1, 128], f16, name="one_sb")
        gam_sb = consts.tile([128, 1], f32, name="gam_sb")
        exp_warm = consts.tile([1, 128], f32, name="exp_warm")
        for h in range(2):
            nc.sync.dma_start(out=wq_sb[:, h * 128:(h + 1) * 128],
                              in_=wq_d[h * 128:(h + 1) * 128, :])
            nc.sync.dma_start(out=wk_sb[:, h * 128:(h + 1) * 128],
                              in_=wk_d[h * 128:(h + 1) * 128, :])
            nc.sync.dma_start(out=wv_sb[:, h * 256:(h + 1) * 256],
                              in_=wv_d[h * 128:(h + 1) * 128, :])
        nc.sync.dma_start(out=bq_sb, in_=bq_d)
        nc.sync.dma_start(out=bk_sb, in_=bk_d)
        nc.sync.dma_start(out=bv_sb, in_=bv_d)
        nc.sync.dma_start(out=gam_sb, in_=gam_d)
        nc.sync.dma_start(out=one_sb, in_=one_d)

        # ---- persistent tiles ----
        q_rep = big.tile([128, N], f16, name="q_rep")
        k_rep = big.tile([128, N], f16, name="k_rep")
        vt = big.tile([128, NTT * C], bf16, name="vt")
        acc_sb = big.tile([128, 2 * N], f32, name="acc_sb")
        zmbig = big.tile([128, 2 * N], f16, name="zmbig")

        def zm_top(g):
            return zmbig[:, g * 256:(g + 1) * 256]

        def zm_bot(g):
            return zmbig[:, N + g * 256:N + (g + 1) * 256]

        for _rep in range(repeat):
          # ACT exp table preload, overlapping the DMAs
          nc.scalar.activation(exp_warm, one_sb, AF.Exp)
          # zm first (stage-1 needs it immediately), zc afterwards on SWDGE;
          # first quarter split finer so k quad 0 starts ~1.4us earlier
          if lesion != "purexp":
            for hf in (0, 1, 2, 3):
              nc.sync.dma_start(out=zmbig[:, hf * 1024:(hf + 1) * 1024],
                                in_=zm_d[0:128, hf * 1024:(hf + 1) * 1024])
              nc.gpsimd.dma_start(out=zmbig[:, N + hf * 1024:N + (hf + 1) * 1024],
                                  in_=zm_d[128:256, hf * 1024:(hf + 1) * 1024])
            for h in range(2):
              for qtr in range(2):
                nc.gpsimd.dma_start(
                    out=acc_sb[:, h * N + qtr * 2048:h * N + (qtr + 1) * 2048],
                    in_=zc_d[h * 128:(h + 1) * 128, qtr * 2048:(qtr + 1) * 2048])
          else:
            nc.vector.memset(q_rep, 0.01)
            nc.vector.memset(k_rep, 0.01)

          def emit_k_quad(qd, on_act):
            psk = ps_c.tile([128, 1024], f32, name="psk", tag="c")
            for j in range(4):
                g = 4 * qd + j
                nc.tensor.matmul(psk[:, j * 256:(j + 1) * 256], wk_sb[:, 0:128],
                                 zm_top(g), start=True, stop=False)
                nc.tensor.matmul(psk[:, j * 256:(j + 1) * 256], wk_sb[:, 128:256],
                                 zm_bot(g), start=False, stop=True)
            dst = k_rep[:, qd * 1024:(qd + 1) * 1024]
            if on_act:
                nc.scalar.activation(dst, psk, AF.Identity, bias=bk_sb)
            else:
                nc.vector.tensor_scalar_add(dst, psk, bk_sb)

          def emit_q_quad(qd):
            psq = ps_c.tile([128, 1024], f32, name="psq", tag="c")
            for j in range(4):
                g = 4 * qd + j
                nc.tensor.matmul(psq[:, j * 256:(j + 1) * 256], wq_sb[:, 0:128],
                                 zm_top(g), start=True, stop=False)
                nc.tensor.matmul(psq[:, j * 256:(j + 1) * 256], wq_sb[:, 128:256],
                                 zm_bot(g), start=False, stop=True)
            nc.vector.tensor_scalar_add(q_rep[:, qd * 1024:(qd + 1) * 1024],
                                        psq, bq_sb)

          def emit_v_run(r):
            psv = ps_c.tile([128, 1024], f32, name="psv", tag="c")
            for j in range(2):
                g = 2 * r + j
                for s in range(2):
                    o = j * 512 + s * 256
                    nc.tensor.matmul(psv[:, o:o + 256], one_sb,
                                     bv_sb, start=True, stop=False)
                    nc.tensor.matmul(psv[:, o:o + 256],
                                     zm_top(g)[:, s * 128:(s + 1) * 128],
                                     wv_sb[:, 0:256], start=False, stop=False)
                    nc.tensor.matmul(psv[:, o:o + 256],
                                     zm_bot(g)[:, s * 128:(s + 1) * 128],
                                     wv_sb[:, 256:512], start=False, stop=True)
            nc.vector.tensor_copy(vt[:, r * 4 * C:(r + 1) * 4 * C], psv)

          # prefix: k quads 0-1 (i 0..2047, enough for the h0 units), q quad 0
          if lesion != "purexp":
            emit_k_quad(0, True)
            emit_q_quad(0)
            emit_k_quad(1, True)

          # (run, unit) -> stage-1 leftovers, placed inside PE slack
          lsched = {
              (0, 0): [("k", 2)], (0, 1): [("k", 3)], (0, 2): [("v", 0)],
              (0, 3): [("v", 1)], (0, 4): [("q", 1)], (0, 5): [("v", 2)],
              (0, 6): [("v", 3)], (0, 7): [("q", 2)],
              (2, 2): [("v", 4)], (3, 2): [("v", 5)], (3, 6): [("q", 3)],
              (4, 2): [("v", 6)], (5, 2): [("v", 7)],
          }
          if lesion == "purexp":
            lsched = {}
          emitters = {"k": lambda g: emit_k_quad(g, False),
                      "q": emit_q_quad, "v": emit_v_run}

          pairs = [(c, kk) for kk in range(4) for c in range(2)]

          def emit_chain_pair(run, pts, uts, c, kk):
            if lesion == "purexp":
                return
            a = ps_c.tile([128, 1024], f32, name="a_out", tag="c")
            for tl in range(RUN):
                for j in range(2):
                    nc.tensor.matmul(a[:, j * 512:(j + 1) * 512],
                                     uts[tl][:, c * 128:(c + 1) * 128],
                                     pts[tl][:, kk * 1024 + j * 512:
                                             kk * 1024 + (j + 1) * 512],
                                     start=(tl == 0), stop=(tl == RUN - 1))
            dst = acc_sb[:, c * N + kk * 1024:c * N + (kk + 1) * 1024]
            nc.vector.tensor_tensor(dst, a, dst, op=OP.add)
            if run == NRUN - 1:
                nc.gpsimd.dma_start(
                    out=out_d[c * 128:(c + 1) * 128, kk * 1024:(kk + 1) * 1024],
                    in_=dst)

          UNITS = [(0, 0), (1, 0), (2, 0), (3, 0), (0, 1), (1, 1), (2, 1), (3, 1)]

          prev = None
          for run in range(NRUN):
            pts = [None] * RUN
            uts = [None] * RUN
            dcol = d_pool.tile([128, 4 * RUN], f32, name="dcol", tag="dcol")
            for u, (tl, half) in enumerate(UNITS):
                tt = run * RUN + tl
                if half == 0:
                    pts[tl] = p_pool.tile([128, N], bf16, name="pt", tag="pt")
                for p in range(2):
                    sp = ps_s.tile([128, 1024], f32, name="s_sc", tag="s")
                    for r in range(2 * p, 2 * p + 2):
                        ic = half * 4 + r
                        nc.tensor.matmul(
                            sp[:, (r - 2 * p) * 512:(r - 2 * p + 1) * 512],
                            q_rep[32 * r:32 * (r + 1), tt * TT:(tt + 1) * TT],
                            k_rep[32 * r:32 * (r + 1), ic * IC:(ic + 1) * IC],
                            start=True, stop=True, tile_position=(32 * r, 0))
                    nc.scalar.activation(
                        pts[tl][:, half * 2048 + p * 1024:
                                half * 2048 + (p + 1) * 1024],
                        sp, AF.Exp,
                        accum_out=dcol[:, (tl * 2 + half) * 2 + p:
                                       (tl * 2 + half) * 2 + p + 1])
                if u in (5, 7) and lesion != "purexp":
                    hb = (u - 5) // 2
                    dview = dcol.rearrange("p (t h) -> p t h", h=4)
                    da = d_pool.tile([128, 2], f32, name="da", tag="da")
                    nc.vector.tensor_tensor(da, dview[:, 2 * hb:2 * hb + 2, 0],
                                            dview[:, 2 * hb:2 * hb + 2, 1],
                                            op=OP.add)
                    db = d_pool.tile([128, 2], f32, name="db", tag="db")
                    nc.vector.tensor_tensor(db, dview[:, 2 * hb:2 * hb + 2, 2],
                                            dview[:, 2 * hb:2 * hb + 2, 3],
                                            op=OP.add)
                    dsum = d_pool.tile([128, 2], f32, name="dsum", tag="dsum")
                    nc.vector.tensor_tensor(dsum, da, db, op=OP.add)
                    drec = d_pool.tile([128, 2], f32, name="drec", tag="drec")
                    nc.vector.reciprocal(drec, dsum)
                    for tl2 in (2 * hb, 2 * hb + 1):
                        tt2 = run * RUN + tl2
                        ut = ut_pool.tile([128, C], bf16, name="ut", tag="ut")
                        uts[tl2] = ut
                        nc.vector.tensor_scalar(ut, vt[:, tt2 * C:(tt2 + 1) * C],
                                                drec[:, tl2 - 2 * hb:tl2 - 2 * hb + 1],
                                                gam_sb, op0=OP.mult, op1=OP.mult)
                if prev is not None:
                    emit_chain_pair(run - 1, prev[0], prev[1], *pairs[u])
                for kind, g in lsched.get((run, u), []):
                    emitters[kind](g)
            prev = (pts, uts)
          for u in range(2 * RUN):
            emit_chain_pair(NRUN - 1, prev[0], prev[1], *pairs[u])
          if lesion == "purexp":
            nc.gpsimd.dma_start(out=out_d[0:128, :], in_=acc_sb[:, 0:N])

    nc.compile()
    return nc


def _get_module(repeat=1, lesion=None):
    key = f"nc{repeat}_{lesion}"
    if key not in _cache:
        _cache[key] = _build_module(repeat, lesion)
    return _cache[key]


def _host_prep(Wq, bq, Wk, bk, Wv, bv, gamma):
    g = np.float32(np.asarray(gamma).reshape(-1)[0])
    wq4 = np.ascontiguousarray(np.tile(np.asarray(Wq).T.astype(np.float16), (1, 4)))
    wk4 = np.ascontiguousarray(np.tile(np.asarray(Wk).T.astype(np.float16), (1, 4)))
    wvt = np.ascontiguousarray(np.asarray(Wv).T.astype(np.float16))
    bq4 = np.ascontiguousarray(np.tile(np.asarray(bq).astype(np.float32), 4).reshape(128, 1))
    bk4 = np.ascontiguousarray(np.tile(np.asarray(bk).astype(np.float32), 4).reshape(128, 1))
    bvr = np.ascontiguousarray(np.asarray(bv).astype(np.float16).reshape(1, C))
    gam = np.full((128, 1), g, np.float32)
    ones = np.ones((1, 128), np.float16)
    return dict(wq4=wq4, wk4=wk4, wvt=wvt, bq4=bq4, bk4=bk4, bvr=bvr, gam=gam, ones=ones)


def kernel(zc, zm, Wq, bq, Wk, bk, Wv, bv, gamma):
    from concourse.bass_utils import run_bass_kernel_spmd

    zc = np.asarray(zc)
    zm = np.asarray(zm)
    b, c, w, h = zm.shape
    assert (b, c, w * h) == (B, C, N), (zm.shape,)
    nc = _get_module()
    shared = _host_prep(Wq, bq, Wk, bk, Wv, bv, gamma)
    zmf = np.ascontiguousarray(zm.reshape(B, C, N).astype(np.float16))
    zcf = np.ascontiguousarray(zc.reshape(B, C, N).astype(np.float32))
    in_maps = [dict(zm=zmf[i], zc=zcf[i], **shared) for i in range(B)]
    res = run_bass_kernel_spmd(nc, in_maps, core_ids=list(range(B)))
    out = np.stack([r["out"] for r in res.results], axis=0)
    return out.reshape(b, c, w, h).astype(np.asarray(zc).dtype)
